# revision 28
# baseline (speedup 1.0000x reference)
"""ArcFace loss on 8 TRN2 NeuronCores — class-parallel (vocab-sharded).

Math: loss = mean_b[ M0 + ln(Z'_b) - s*phi_b ] with
  Z_b  = sum_c exp(s*cos(b,c) - M0)          (device, sharded over classes)
  Z'_b = Z_b - exp(s*cos(b,l_b) - M0) + exp(s*phi_b - M0)   (label correction)
M0 is a fixed logsumexp shift: |cos| <= 1 so s*cos - M0 <= 80 and
exp() can never overflow f32 (e^80 < f32 max); on the real data
|s*cos| <= ~36, so all terms stay in the normal f32 range.

Host (inside kernel()): row-normalize x and W, fold nothing into W, transpose
W shards to [D, C_shard] so the device needs no on-chip transposes, and
evaluate the tiny label/phi terms ([512] vectors). Device: the 512x512x100k
matmul, exp, row-sums, an AllGather of per-core partial Z, and the final
scalar reduction.
"""

import math

import numpy as np

from concourse import bacc, mybir
from concourse.bass_utils import run_bass_kernel_spmd
from concourse.tile import TileContext

NCORES = 8
B = 512
D = 512
C = 100000
CS = 12544  # per-core classes, padded: 8 * 12544 = 100352 >= C
S = 120.0
MARGIN = 0.3
COS_M = math.cos(MARGIN)
SIN_M = math.sin(MARGIN)
TH = math.cos(math.pi - MARGIN)
MM = math.sin(math.pi - MARGIN) * MARGIN
M0 = 40.0  # logsumexp shift
SUPER = 2048  # class columns per DMA (1 MiB per [128, 2048] f32 tile)
NBLK = 512  # class columns per matmul (one PSUM bank)

F32 = mybir.dt.float32
BF16 = mybir.dt.bfloat16
FN = mybir.ActivationFunctionType

_GRAPH = None
LAST_RESULT = None  # BassKernelResults of the most recent run (for test harness)


def _build_nc(repeat=1):
    """Build the SPMD graph. repeat>1 unrolls the whole body N times into one
    NEFF (timing only: amortizes the per-execute dispatch overhead)."""
    nc = bacc.Bacc("TRN2", target_bir_lowering=False)

    # const AP for the Exp bias (only 0.0/1.0 are pre-registered)
    _cb = nc.alloc_sbuf_tensor(f"const-float32-{-M0}", [128, 1], F32)
    nc.gpsimd.memset(_cb.ap(), -M0)
    nc.const_aps.aps[(F32, -M0)] = _cb.ap()
    nc.all_engine_barrier()

    xt = nc.declare_dram_parameter("xt", [D, B], BF16, isOutput=False)
    wt = nc.declare_dram_parameter("wt", [D, CS], BF16, isOutput=False)
    sl = nc.declare_dram_parameter("sl", [1, B], F32, isOutput=False)
    su = nc.declare_dram_parameter("su", [1, B], F32, isOutput=False)
    out = nc.declare_dram_parameter("out", [1, 1], F32, isOutput=True)

    with TileContext(nc, num_cores=NCORES) as tc:
        with (
            tc.tile_pool(name="xpool", bufs=1) as xpool,
            tc.tile_pool(name="wpool", bufs=3) as wpool,
            tc.tile_pool(name="epool", bufs=3) as epool,
            tc.tile_pool(name="zpool", bufs=1) as zpool,
            tc.tile_pool(name="spool", bufs=1) as spool,
            tc.tile_pool(name="psum", bufs=2, space="PSUM") as pp,
            tc.tile_pool(name="dram", bufs=1, space="DRAM") as dram,
        ):
            # x^T (normalized, transposed on host): 4 contraction chunks [128, B]
            xts = []
            for k in range(4):
                t = xpool.tile([128, B], BF16, tag=f"xt{k}", name=f"xts{k}")
                nc.sync.dma_start(t[:], xt[k * 128 : (k + 1) * 128, :])
                xts.append(t)

            sl_sb = spool.tile([1, B], F32, tag="sl")
            nc.sync.dma_start(sl_sb[:], sl[:])
            su_sb = spool.tile([1, B], F32, tag="su")
            nc.sync.dma_start(su_sb[:], su[:])

            # label-correction terms: computed up front, hidden under main loop
            t1 = spool.tile([1, B], F32, tag="t1")
            nc.scalar.activation(t1[:], sl_sb[:], FN.Exp, bias=-M0, scale=1.0)
            t2 = spool.tile([1, B], F32, tag="t2")
            nc.scalar.activation(t2[:], su_sb[:], FN.Exp, bias=-M0, scale=1.0)

            for rep in range(repeat):
                _body(nc, tc, rep, xpool, wpool, epool, zpool, spool, pp, dram,
                      xt, wt, out, xts, sl_sb, su_sb, t1, t2)

    if not nc.is_finalized():
        nc.finalize()
    return nc


def _body(nc, tc, rep, xpool, wpool, epool, zpool, spool, pp, dram,
          xt, wt, out, xts, sl_sb, su_sb, t1, t2):
    # per-batch-tile accumulators of per-block exp-sums (one col/block)
    zbufs = [
        zpool.tile([128, 32], F32, tag=f"zb{bi}", name=f"zb{bi}_{rep}")
        for bi in range(4)
    ]

    # superblock widths: small leading chunks let the first matmuls start
    # before the bulk DMAs land; 256-tail absorbs the ragged remainder
    sws = [512, 512, 1024] + [SUPER] * ((CS - 2304) // SUPER)
    sws.append(CS - sum(sws))
    assert sum(sws) == CS and all(w > 0 for w in sws)

    col = 0
    c0 = 0
    for sw in sws:
        wts = []
        for k in range(4):
            t = wpool.tile([128, SUPER], BF16, tag=f"w{k}", name=f"wts{k}_{rep}")
            nc.sync.dma_start(
                t[:, :sw], wt[k * 128 : (k + 1) * 128, c0 : c0 + sw]
            )
            wts.append(t)
        for bi in range(4):
            # [128, SUPER] f32 psum tile = 4 banks; bufs=2 -> all 8
            ps = pp.tile([128, SUPER], F32, tag="ps", name=f"ps_{rep}")
            nb0 = 0
            while nb0 < sw:
                nb = min(NBLK, sw - nb0)
                for k in range(4):
                    nc.tensor.matmul(
                        ps[:, nb0 : nb0 + nb],
                        xts[k][:, bi * 128 : (bi + 1) * 128],
                        wts[k][:, nb0 : nb0 + nb],
                        start=(k == 0),
                        stop=(k == 3),
                    )
                nb0 += nb
            # one big exp per (superblock, batch-tile): PSUM -> SBUF scratch
            # (in-place PSUM exp measured ~5x slower on HW); only the
            # accum_out row-sum is consumed downstream
            ex = epool.tile([128, SUPER], F32, tag="ex", name=f"ex_{rep}")
            nc.scalar.activation(
                ex[:, :sw],
                ps[:, :sw],
                FN.Exp,
                bias=-M0,
                scale=S,
                accum_out=zbufs[bi][:, col : col + 1],
            )
        col += 1
        c0 += sw
    ncol = col  # number of superblocks

    # partial Z per core -> DRAM [B] in natural batch order
    zdram = dram.tile([B], F32, name=f"zdram_{rep}")
    for bi in range(4):
        zs = zpool.tile([128, 1], F32, tag=f"zs{bi}", name=f"zs{bi}_{rep}")
        nc.vector.reduce_sum(
            zs[:], zbufs[bi][:, :ncol], axis=mybir.AxisListType.X
        )
        nc.sync.dma_start(zdram[bi * 128 : (bi + 1) * 128], zs[:])

    zgat = dram.tile([NCORES * B], F32, name=f"zgat_{rep}")
    nc.gpsimd.collective_compute(
        "AllGather",
        mybir.AluOpType.bypass,
        replica_groups=[list(range(NCORES))],
        ins=[zdram.opt()],
        outs=[zgat.opt()],
    )

    # cross-core sum of the gathered partials on the PE:
    # zsum[1, B] = ones[1, 8] @ zg[8, B]
    zg = spool.tile([NCORES, B], F32, tag="zg", name=f"zg_{rep}")
    nc.sync.dma_start(zg[:], zgat.rearrange("(r b) -> r b", r=NCORES))
    ones = nc.const_aps.aps[(F32, 1.0)]
    zps = pp.tile([128, SUPER], F32, tag="ps", name=f"zps_{rep}")
    nc.tensor.matmul(
        zps[:1, :B], ones[:NCORES, :1], zg[:], start=True, stop=True
    )

    # label correction + final scalar
    zc = spool.tile([1, B], F32, tag="zc", name=f"zc_{rep}")
    nc.vector.tensor_sub(zc[:], zps[:1, :B], t1[:])
    nc.vector.tensor_add(zc[:], zc[:], t2[:])
    lg = spool.tile([1, B], F32, tag="lg", name=f"lg_{rep}")
    nc.scalar.activation(lg[:], zc[:], FN.Ln)
    v = spool.tile([1, B], F32, tag="v", name=f"v_{rep}")
    nc.vector.tensor_sub(v[:], lg[:], su_sb[:])
    r = spool.tile([1, 1], F32, tag="r", name=f"r_{rep}")
    nc.vector.reduce_sum(r[:], v[:], axis=mybir.AxisListType.X)
    ov = spool.tile([1, 1], F32, tag="ov", name=f"ov_{rep}")
    nc.scalar.activation(ov[:], r[:], FN.Copy, bias=M0, scale=1.0 / B)
    nc.sync.dma_start(out[:], ov[:])


def _host_prep(input, label, weight):
    x = np.asarray(input, dtype=np.float32)
    lab = np.asarray(label).astype(np.int64).ravel()
    w = np.asarray(weight, dtype=np.float32)

    xn64 = x.astype(np.float64)
    xn64 /= np.maximum(
        np.sqrt(np.einsum("bd,bd->b", xn64, xn64))[:, None], 1e-12
    )
    bf16 = mybir.dt.np(BF16)
    xt = np.ascontiguousarray(xn64.T.astype(np.float32)).astype(bf16)  # [D, B]

    wn_inv = 1.0 / np.maximum(
        np.sqrt(np.einsum("cd,cd->c", w, w, dtype=np.float64)), 1e-12
    )
    wn = w * wn_inv[:, None].astype(np.float32)  # [C, D] normalized rows, f32

    # label terms (tiny, f64)
    wl = wn[lab].astype(np.float64)  # [B, D]
    cosl = np.einsum("bd,bd->b", xn64, wl)
    cosl = np.clip(cosl, -1.0, 1.0)
    sine = np.sqrt(np.maximum(1.0 - cosl * cosl, 0.0))
    phi = cosl * COS_M - sine * SIN_M
    phi = np.where(cosl > TH, phi, cosl - MM)
    sl = (S * cosl).astype(np.float32).reshape(1, B)
    su = (S * phi).astype(np.float32).reshape(1, B)

    # class-sharded, transposed W: [D, CS] per core, zero-padded at the tail
    shards = []
    for i in range(NCORES):
        lo, hi = i * CS, min((i + 1) * CS, C)
        sh = np.zeros((D, CS), dtype=bf16)
        sh[:, : hi - lo] = wn[lo:hi].T.astype(bf16)
        shards.append(np.ascontiguousarray(sh))
    return xt, sl, su, shards


def kernel(input, label, weight):
    global _GRAPH, LAST_RESULT
    xt, sl, su, shards = _host_prep(input, label, weight)
    if _GRAPH is None:
        _GRAPH = _build_nc()
    in_maps = [
        {"xt": xt, "wt": shards[i], "sl": sl, "su": su} for i in range(NCORES)
    ]
    res = run_bass_kernel_spmd(_GRAPH, in_maps, list(range(NCORES)))
    LAST_RESULT = res
    outv = np.asarray(res.results[0]["out"], dtype=np.float32)
    return outv.reshape(())


# revision 30
# speedup vs baseline: 9.0108x; 9.0108x over previous
"""ArcFace loss on 8 TRN2 NeuronCores — class-parallel (vocab-sharded).

Math: loss = mean_b[ M0 + ln(Z'_b) - s*phi_b ] with
  Z_b  = sum_c exp(s*cos(b,c) - M0)          (device, sharded over classes)
  Z'_b = Z_b - exp(s*cos(b,l_b) - M0) + exp(s*phi_b - M0)   (label correction)
M0 is a fixed logsumexp shift: |cos| <= 1 so s*cos - M0 <= 80 and
exp() can never overflow f32 (e^80 < f32 max); on the real data
|s*cos| <= ~36, so all terms stay in the normal f32 range.

Host (inside kernel()): row-normalize x and W, fold nothing into W, transpose
W shards to [D, C_shard] so the device needs no on-chip transposes, and
evaluate the tiny label/phi terms ([512] vectors). Device: the 512x512x100k
matmul, exp, row-sums, an AllGather of per-core partial Z, and the final
scalar reduction.
"""

import math

import numpy as np

from concourse import bacc, mybir
from concourse.bass_utils import run_bass_kernel_spmd
from concourse.tile import TileContext

NCORES = 8
B = 512
D = 512
C = 100000
CS = 12544  # per-core classes, padded: 8 * 12544 = 100352 >= C
S = 120.0
MARGIN = 0.3
COS_M = math.cos(MARGIN)
SIN_M = math.sin(MARGIN)
TH = math.cos(math.pi - MARGIN)
MM = math.sin(math.pi - MARGIN) * MARGIN
M0 = 40.0  # logsumexp shift
SUPER = 2048  # class columns per DMA (1 MiB per [128, 2048] f32 tile)
NBLK = 512  # class columns per matmul (one PSUM bank)

F32 = mybir.dt.float32
BF16 = mybir.dt.bfloat16
FN = mybir.ActivationFunctionType

_GRAPH = None
LAST_RESULT = None  # BassKernelResults of the most recent run (for test harness)


def _build_nc(repeat=1):
    """Build the SPMD graph. repeat>1 unrolls the whole body N times into one
    NEFF (timing only: amortizes the per-execute dispatch overhead)."""
    nc = bacc.Bacc("TRN2", target_bir_lowering=False)

    # const AP for the Exp bias (only 0.0/1.0 are pre-registered)
    _cb = nc.alloc_sbuf_tensor(f"const-float32-{-M0}", [128, 1], F32)
    nc.gpsimd.memset(_cb.ap(), -M0)
    nc.const_aps.aps[(F32, -M0)] = _cb.ap()
    nc.all_engine_barrier()

    xt = nc.declare_dram_parameter("xt", [D, B], BF16, isOutput=False)
    wt = nc.declare_dram_parameter("wt", [D, CS], BF16, isOutput=False)
    sl = nc.declare_dram_parameter("sl", [1, B], F32, isOutput=False)
    su = nc.declare_dram_parameter("su", [1, B], F32, isOutput=False)
    out = nc.declare_dram_parameter("out", [1, 1], F32, isOutput=True)

    with TileContext(nc, num_cores=NCORES) as tc:
        with (
            tc.tile_pool(name="xpool", bufs=1) as xpool,
            tc.tile_pool(name="wpool", bufs=3) as wpool,
            tc.tile_pool(name="epool", bufs=3) as epool,
            tc.tile_pool(name="zpool", bufs=1) as zpool,
            tc.tile_pool(name="spool", bufs=1) as spool,
            tc.tile_pool(name="psum", bufs=2, space="PSUM") as pp,
            tc.tile_pool(name="dram", bufs=1, space="DRAM") as dram,
        ):
            # x^T (normalized, transposed on host): 4 contraction chunks [128, B]
            xts = []
            for k in range(4):
                t = xpool.tile([128, B], BF16, tag=f"xt{k}", name=f"xts{k}")
                nc.sync.dma_start(t[:], xt[k * 128 : (k + 1) * 128, :])
                xts.append(t)

            sl_sb = spool.tile([1, B], F32, tag="sl")
            nc.sync.dma_start(sl_sb[:], sl[:])
            su_sb = spool.tile([1, B], F32, tag="su")
            nc.sync.dma_start(su_sb[:], su[:])

            # label-correction terms: computed up front, hidden under main loop
            t1 = spool.tile([1, B], F32, tag="t1")
            nc.scalar.activation(t1[:], sl_sb[:], FN.Exp, bias=-M0, scale=1.0)
            t2 = spool.tile([1, B], F32, tag="t2")
            nc.scalar.activation(t2[:], su_sb[:], FN.Exp, bias=-M0, scale=1.0)

            for rep in range(repeat):
                _body(nc, tc, rep, xpool, wpool, epool, zpool, spool, pp, dram,
                      xt, wt, out, xts, sl_sb, su_sb, t1, t2)

    if not nc.is_finalized():
        nc.finalize()
    return nc


def _body(nc, tc, rep, xpool, wpool, epool, zpool, spool, pp, dram,
          xt, wt, out, xts, sl_sb, su_sb, t1, t2):
    # per-batch-tile accumulators of per-block exp-sums (one col/block)
    zbufs = [
        zpool.tile([128, 32], F32, tag=f"zb{bi}", name=f"zb{bi}_{rep}")
        for bi in range(4)
    ]

    # superblock widths: small leading chunks let the first matmuls start
    # before the bulk DMAs land; 256-tail absorbs the ragged remainder
    sws = [512, 512, 1024] + [SUPER] * ((CS - 2304) // SUPER)
    sws.append(CS - sum(sws))
    assert sum(sws) == CS and all(w > 0 for w in sws)

    col = 0
    c0 = 0
    for sw in sws:
        wts = []
        for k in range(4):
            t = wpool.tile([128, SUPER], BF16, tag=f"w{k}", name=f"wts{k}_{rep}")
            nc.sync.dma_start(
                t[:, :sw], wt[k * 128 : (k + 1) * 128, c0 : c0 + sw]
            )
            wts.append(t)
        for bi in range(4):
            # [128, SUPER] f32 psum tile = 4 banks; bufs=2 -> all 8
            ps = pp.tile([128, SUPER], F32, tag="ps", name=f"ps_{rep}")
            nb0 = 0
            while nb0 < sw:
                nb = min(NBLK, sw - nb0)
                for k in range(4):
                    nc.tensor.matmul(
                        ps[:, nb0 : nb0 + nb],
                        xts[k][:, bi * 128 : (bi + 1) * 128],
                        wts[k][:, nb0 : nb0 + nb],
                        start=(k == 0),
                        stop=(k == 3),
                    )
                nb0 += nb
            # one big exp per (superblock, batch-tile): PSUM -> SBUF scratch
            # (in-place PSUM exp measured ~5x slower on HW); only the
            # accum_out row-sum is consumed downstream
            ex = epool.tile([128, SUPER], F32, tag="ex", name=f"ex_{rep}")
            nc.scalar.activation(
                ex[:, :sw],
                ps[:, :sw],
                FN.Exp,
                bias=-M0,
                scale=S,
                accum_out=zbufs[bi][:, col : col + 1],
            )
        col += 1
        c0 += sw
    ncol = col  # number of superblocks

    # partial Z per core -> DRAM [B] in natural batch order
    zdram = dram.tile([B], F32, name=f"zdram_{rep}")
    for bi in range(4):
        zs = zpool.tile([128, 1], F32, tag=f"zs{bi}", name=f"zs{bi}_{rep}")
        nc.vector.reduce_sum(
            zs[:], zbufs[bi][:, :ncol], axis=mybir.AxisListType.X
        )
        nc.sync.dma_start(zdram[bi * 128 : (bi + 1) * 128], zs[:])

    zgat = dram.tile([NCORES * B], F32, name=f"zgat_{rep}")
    nc.gpsimd.collective_compute(
        "AllGather",
        mybir.AluOpType.bypass,
        replica_groups=[list(range(NCORES))],
        ins=[zdram.opt()],
        outs=[zgat.opt()],
    )

    # cross-core sum of the gathered partials on the PE:
    # zsum[1, B] = ones[1, 8] @ zg[8, B]
    zg = spool.tile([NCORES, B], F32, tag="zg", name=f"zg_{rep}")
    nc.sync.dma_start(zg[:], zgat.rearrange("(r b) -> r b", r=NCORES))
    ones = nc.const_aps.aps[(F32, 1.0)]
    zps = pp.tile([128, SUPER], F32, tag="ps", name=f"zps_{rep}")
    nc.tensor.matmul(
        zps[:1, :B], ones[:NCORES, :1], zg[:], start=True, stop=True
    )

    # label correction + final scalar
    zc = spool.tile([1, B], F32, tag="zc", name=f"zc_{rep}")
    nc.vector.tensor_sub(zc[:], zps[:1, :B], t1[:])
    nc.vector.tensor_add(zc[:], zc[:], t2[:])
    lg = spool.tile([1, B], F32, tag="lg", name=f"lg_{rep}")
    nc.scalar.activation(lg[:], zc[:], FN.Ln)
    v = spool.tile([1, B], F32, tag="v", name=f"v_{rep}")
    nc.vector.tensor_sub(v[:], lg[:], su_sb[:])
    r = spool.tile([1, 1], F32, tag="r", name=f"r_{rep}")
    nc.vector.reduce_sum(r[:], v[:], axis=mybir.AxisListType.X)
    ov = spool.tile([1, 1], F32, tag="ov", name=f"ov_{rep}")
    nc.scalar.activation(ov[:], r[:], FN.Copy, bias=M0, scale=1.0 / B)
    nc.sync.dma_start(out[:], ov[:])


def _host_prep(input, label, weight):
    x = np.asarray(input, dtype=np.float32)
    lab = np.asarray(label).astype(np.int64).ravel()
    w = np.asarray(weight, dtype=np.float32)

    xn64 = x.astype(np.float64)
    xn64 /= np.maximum(
        np.sqrt(np.einsum("bd,bd->b", xn64, xn64))[:, None], 1e-12
    )
    bf16 = mybir.dt.np(BF16)
    xt = np.ascontiguousarray(xn64.T.astype(np.float32)).astype(bf16)  # [D, B]

    wn_inv = 1.0 / np.maximum(
        np.sqrt(np.einsum("cd,cd->c", w, w, dtype=np.float64)), 1e-12
    )
    wn = w * wn_inv[:, None].astype(np.float32)  # [C, D] normalized rows, f32

    # label terms (tiny, f64)
    wl = wn[lab].astype(np.float64)  # [B, D]
    cosl = np.einsum("bd,bd->b", xn64, wl)
    cosl = np.clip(cosl, -1.0, 1.0)
    sine = np.sqrt(np.maximum(1.0 - cosl * cosl, 0.0))
    phi = cosl * COS_M - sine * SIN_M
    phi = np.where(cosl > TH, phi, cosl - MM)
    sl = (S * cosl).astype(np.float32).reshape(1, B)
    su = (S * phi).astype(np.float32).reshape(1, B)

    # class-sharded, transposed W: [D, CS] per core, zero-padded at the tail
    shards = []
    for i in range(NCORES):
        lo, hi = i * CS, min((i + 1) * CS, C)
        sh = np.zeros((D, CS), dtype=bf16)
        sh[:, : hi - lo] = wn[lo:hi].T.astype(bf16)
        shards.append(np.ascontiguousarray(sh))
    return xt, sl, su, shards


def kernel(input, label, weight):
    global _GRAPH, LAST_RESULT
    xt, sl, su, shards = _host_prep(input, label, weight)
    if _GRAPH is None:
        _GRAPH = _build_nc()
    in_maps = [
        {"xt": xt, "wt": shards[i], "sl": sl, "su": su} for i in range(NCORES)
    ]
    res = run_bass_kernel_spmd(_GRAPH, in_maps, list(range(NCORES)))
    LAST_RESULT = res
    outv = np.asarray(res.results[0]["out"], dtype=np.float32)
    return outv.reshape(())
